# revision 39
# baseline (speedup 1.0000x reference)
"""AttnPool Trainium2 kernel (nn_AttnPool_73100343378373).

Math (algebraically identical to the reference):
    scores = (q @ w) @ x.T   per batch  -> (H, L)
    attn   = softmax(scores + mask_bias, axis=L)
    out    = attn @ x  -> (B, H*D)

Distribution: data-parallel over batch, 2 batches per core, q/w replicated.

Precision scheme (fp8 e4m3 on the whole hot path):
  - x is split host-side into three fp8 planes: x ~= x1 + x2s/64 + x3s/64.
  - qw = q@w is computed on device in fp32 (w-stationary matmuls so the
    fp32 4x row cost applies to 8 moving columns only), split into three
    fp8 planes (residual chain) and folded into two 24-wide (padded to
    32) DoubleRow stationaries: S1 = [q1|q2|q3] for the x1 pass, S2 =
    S1/64 for the x2s/x3s passes.  Three DoubleRow score passes per
    group accumulate all 9 cross terms into the same PSUM rows.
  - softmax is near-one-hot (score std ~1200 over L=4096); u = exp fp8.
  - pooled = u @ x1 via DoubleRow matmuls on a pair-layout derived
    on-device from the x1 score plane (bit-exact bf16-bitcast PE
    transposes), plus an exact top-1 correction: the argmax row's
    residual r2 = x - f32(x1) is fetched with an indirect DMA gather
    keyed by on-device max_with_indices, added before normalization.

Scheduling notes (all verified against perfetto traces):
  - ISA: engine (Act/DVE) access patterns must start at 32-aligned
    partitions -> the 3x8-row score PSUM block is realigned via small
    SBUF-to-SBUF DMAs.
  - matmul start=True zeroes the full 2KB PSUM bank for the written
    partitions -> sibling accumulation chains pre-zero with a DVE
    memset and run start=False.
  - SBUF/PSUM pools are created steady-first so the transient stage-0
    pools alias only regions whose first use already depends on
    stage-0 (otherwise the slot-ring DMAs inherit a stage-0 wait).
  - The x1 plane streams in its own slot ring so its pair-transposes
    (which need no stationaries) keep the PE busy during stage 0.
"""

import os
from contextlib import ExitStack

import numpy as np

B, L, D, H = 16, 4096, 1024, 8
NCORES = 8
BPC = B // NCORES  # batches per core
NG = 8             # 512-row L-groups per batch
GL = L // NG       # rows per group = 512
NCC = D // 256     # 256-deep contraction chunks over D = 4
NT = L // 256      # 256-deep contraction chunks over L = 16

VARIANT = {
    "slotA_bufs": 5,
    "slotB_bufs": 3,
    "pd_bufs": 2,
    "tp_bufs": 2,
    "sp_bufs": 3,
    "early_tp": 4,   # groups of batch 0 whose transposes run during stage 0
}

_CACHE: dict = {}
LAST_RESULTS = None  # test harness reads exec_time_ns from here


def _build(masked: bool, variant: dict | None = None):
    import concourse.bass as bass
    import concourse.tile as tile
    from concourse import bacc, mybir
    from concourse.masks import make_identity

    v = dict(VARIANT)
    if variant:
        v.update(variant)
    if masked:
        v["slotA_bufs"] = 4
        v["early_tp"] = 3
    ETP = v["early_tp"]

    f32 = mybir.dt.float32
    fp8 = mybir.dt.float8e4
    bf16 = mybir.dt.bfloat16
    u32 = mybir.dt.uint32
    AF = mybir.ActivationFunctionType
    AX = mybir.AxisListType
    DR = mybir.MatmulPerfMode.DoubleRow

    nc = bacc.Bacc("TRN2", target_bir_lowering=False, debug=False)

    # sd[b, g, p, (pl, cc, r*512 + l)] = xpl[b, 512 g + l, 256 cc + 128 r + p]
    # per-partition bytes contiguous; plane 0 (bytes 0:4096) is the A-stream.
    sd_d = nc.dram_tensor("sd", (BPC, NG, 128, 3 * NCC * 1024), fp8,
                          kind="ExternalInput").ap()
    qT_d = nc.dram_tensor("qT", (D, H), f32, kind="ExternalInput").ap()
    w_d = nc.dram_tensor("w", (D, D), f32, kind="ExternalInput").ap()
    r2_d = [nc.dram_tensor(f"r2_{b}", (L, D), f32, kind="ExternalInput").ap()
            for b in range(BPC)]
    if masked:
        mb_d = nc.dram_tensor("mb", (BPC, H, L), f32, kind="ExternalInput").ap()
    out_d = nc.dram_tensor("out", (BPC, H, D), f32, kind="ExternalOutput").ap()

    with tile.TileContext(nc) as tc, ExitStack() as ctx:
        const = ctx.enter_context(tc.tile_pool(name="const", bufs=1))

        ident = const.tile([128, 128], bf16, tag="ident")
        make_identity(nc, ident[:])
        ident8 = const.tile([128, 128], fp8, tag="ident8")
        nc.vector.tensor_copy(ident8[:], ident[:])
        identf = const.tile([128, 128], f32, tag="identf")
        nc.vector.tensor_copy(identf[:], ident[:])

        # steady pools first (see scheduling notes)
        slotA = ctx.enter_context(tc.tile_pool(name="slotA", bufs=v["slotA_bufs"]))
        slotB = ctx.enter_context(tc.tile_pool(name="slotB", bufs=v["slotB_bufs"]))
        pdp = ctx.enter_context(tc.tile_pool(name="pd", bufs=v["pd_bufs"]))
        scp = ctx.enter_context(tc.tile_pool(name="sc", bufs=2))
        smallp = ctx.enter_context(tc.tile_pool(name="small", bufs=2))
        tpp = ctx.enter_context(tc.tile_pool(name="tp", bufs=v["tp_bufs"], space="PSUM"))
        utpp = ctx.enter_context(tc.tile_pool(name="utps", bufs=1, space="PSUM"))

        st_tiles = [[None] * NCC for _ in range(2)]
        state = {}

        def prep(b, early_tp):
            """Emit the batch's slot DMAs (A then B per group), the uT pad
            zeroing, and optionally the first groups' pair-transposes."""
            pd1 = pdp.tile([128, NT * 2048], fp8, tag="pd1")
            uT = smallp.tile([128, NT * 64], fp8, tag="uT")
            nc.gpsimd.memset(uT[:], 0.0)
            sA, sB = [], []
            for g in range(NG):
                a = slotA.tile([128, NCC * 1024], fp8, tag="slotA")
                nc.sync.dma_start(a[:], sd_d[b, g, :, 0: NCC * 1024])
                bt = slotB.tile([128, 2 * NCC * 1024], fp8, tag="slotB")
                nc.scalar.dma_start(bt[:], sd_d[b, g, :, NCC * 1024:])
                sA.append(a)
                sB.append(bt)
            state[b] = dict(pd1=pd1, uT=uT, sA=sA, sB=sB)
            for g in range(early_tp):
                emit_tp(b, g)

        def emit_tp(b, g):
            """Pair-transpose the x1 plane of group g into the pooled DR
            layout (bf16 bitcast, bit-exact)."""
            pd1 = state[b]["pd1"]
            a = state[b]["sA"][g]
            tv = a[:].rearrange("p (c r t y) -> p c r t y", c=NCC, r=2, t=2)
            for t in range(2):
                tp = tpp.tile([128, 1024], bf16, tag="tp")
                for cc in range(NCC):
                    for r in range(2):
                        nc.tensor.transpose(
                            tp[:, 128 * (2 * cc + r): 128 * (2 * cc + r + 1)],
                            tv[:, cc, r, t].bitcast(bf16),
                            ident[:],
                        )
                dst = pd1[:].bitcast(bf16).rearrange(
                    "p (T y) -> p T y", T=NT)[:, 2 * g + t]
                if (g + t) % 2 == 0:
                    nc.scalar.copy(dst, tp[:])
                else:
                    nc.vector.tensor_copy(dst, tp[:])

        # ---- stage 0: qw = q @ w, plane split, DR stationaries ----------
        with tc.tile_pool(name="s0", bufs=1) as s0p, \
             tc.tile_pool(name="s0w", bufs=2) as s0wp, \
             tc.tile_pool(name="s0ps", bufs=1, space="PSUM") as s0ps:
            qT_sb = const.tile([128, 64], f32, tag="qT")
            nc.gpsimd.dma_start(
                qT_sb[:].rearrange("p (c h) -> p c h", c=8),
                qT_d.rearrange("(c p) h -> p c h", p=128),
            )
            # qw^T via w-stationary matmuls: moving operand is qT (8 cols),
            # so the fp32 4x row cost applies to only 8 columns per matmul.
            qwT_ps = s0ps.tile([128, 64], f32, tag="qwtps")
            nc.vector.memset(qwT_ps[:], 0.0)
            for dc in range(8):
                for wh in range(2):
                    w_t = s0wp.tile([128, 512], f32, tag="w")
                    nc.sync.dma_start(
                        w_t[:], w_d[128 * dc: 128 * (dc + 1),
                                    512 * wh: 512 * (wh + 1)])
                    for fbh in range(4):
                        fb = 4 * wh + fbh
                        nc.tensor.matmul(
                            qwT_ps[:, 8 * fb: 8 * (fb + 1)],
                            w_t[:, 128 * fbh: 128 * (fbh + 1)],
                            qT_sb[:, 8 * dc: 8 * (dc + 1)],
                            start=False, stop=(dc == 7),
                            skip_group_check=True,
                        )

            # batch 0 DMAs + early transposes keep DMA and PE busy while
            # the stationary chain below resolves.
            prep(0, ETP)

            qwT_sb = s0p.tile([128, 64], f32, tag="qwT")
            nc.scalar.copy(qwT_sb[:], qwT_ps[:])
            qw_sb = s0p.tile([8, D], f32, tag="qw")
            for fb in range(8):
                qw2b = s0ps.tile([8, 512], f32, tag="qw2b", bufs=2)
                nc.tensor.transpose(
                    qw2b[:, 0:128],
                    qwT_sb[:, 8 * fb: 8 * (fb + 1)],
                    identf[:],
                )
                nc.scalar.copy(qw_sb[:, 128 * fb: 128 * (fb + 1)],
                               qw2b[:, 0:128])

            # residual-chain plane split, in place: qw_sb becomes the
            # running residual; p8[(0, r)] are the canonical fp8 planes.
            p8 = {}
            for r in range(3):
                for si in range(2):
                    p8[(si, r)] = s0p.tile([8, D], fp8, tag=f"p8_{si}_{r}",
                                           name=f"p8_{si}_{r}")
            q1f = s0p.tile([8, D], f32, tag="q1f")
            nc.scalar.copy(p8[(0, 0)][:], qw_sb[:])
            nc.scalar.copy(q1f[:], p8[(0, 0)][:])
            nc.vector.tensor_sub(qw_sb[:], qw_sb[:], q1f[:])
            nc.scalar.copy(p8[(0, 1)][:], qw_sb[:])
            nc.scalar.copy(q1f[:], p8[(0, 1)][:])
            nc.vector.tensor_sub(qw_sb[:], qw_sb[:], q1f[:])
            nc.scalar.copy(p8[(0, 2)][:], qw_sb[:])
            for r in range(3):
                nc.scalar.activation(p8[(1, r)][:], p8[(0, r)][:],
                                     AF.Copy, scale=1.0 / 64.0)
            for si in range(2):
                for cc in range(NCC):
                    ps = s0ps.tile([128, 2048], fp8, tag="stps", bufs=1)
                    psv = ps[:].rearrange("p (k two) -> p k two", two=2)
                    for i in range(2):
                        for r in range(3):
                            nc.tensor.transpose(
                                psv[:, 32 * i + 8 * r: 32 * i + 8 * r + 8, 0],
                                p8[(si, r)][:, 256 * cc + 128 * i:
                                             256 * cc + 128 * (i + 1)],
                                ident8[0:H, 0:H],
                            )
                    st = const.tile([128, 64], fp8, tag=f"st{si}_{cc}")
                    nc.vector.tensor_copy(st[:], psv[:, 0:64, 0])
                    for i in range(2):
                        nc.gpsimd.memset(st[:, 32 * i + 24: 32 * (i + 1)], 0.0)
                    st_tiles[si][cc] = st

        # PSUM pools that alias the (dead) stage-0 psum: their first use
        # already depends on stage-0 output.
        spp = ctx.enter_context(tc.tile_pool(name="sp", bufs=v["sp_bufs"], space="PSUM"))
        ppp = ctx.enter_context(tc.tile_pool(name="pp", bufs=1, space="PSUM"))

        def compute(b, early_tp):
            if masked:
                mb_sb = scp.tile([H, L], f32, tag="mb", bufs=1)
                nc.gpsimd.dma_start(mb_sb[:], mb_d[b])
            scoresT = scp.tile([H, L], f32, tag="scoresT")
            pmax = smallp.tile([H, NG], f32, tag="pmax")
            state[b].update(scoresT=scoresT, pmax=pmax)
            for g in range(NG):
                a = state[b]["sA"][g]
                bt = state[b]["sB"][g]
                svA = a[:].rearrange("p (c r l) -> p c r l", c=NCC, r=2)
                svB = bt[:].rearrange("p (pl c r l) -> p pl c r l", pl=2, c=NCC, r=2)
                sp = spp.tile([32, GL], f32, tag="sp")
                for pl in range(3):
                    for cc in range(NCC):
                        nc.tensor.matmul(
                            sp[:, :],
                            st_tiles[0 if pl == 0 else 1][cc][:].rearrange(
                                "p (i m) -> p i m", i=2),
                            svA[:, cc] if pl == 0 else svB[:, pl - 1, cc],
                            start=(pl == 0 and cc == 0),
                            stop=(pl == 2 and cc == NCC - 1),
                            perf_mode=DR,
                            skip_group_check=True,
                        )
                if g >= early_tp:
                    emit_tp(b, g)

                # engine APs must start at 32-aligned partitions: copy the
                # 32-row psum block out, then DMA-realign rows 8:24.
                ssp = smallp.tile([32, GL], f32, tag="ssp")
                nc.scalar.copy(ssp[:], sp[:, :])
                spl = smallp.tile([H, 2 * GL], f32, tag="spl")
                nc.gpsimd.dma_start(spl[:, 0:GL], ssp[8:16, :])
                nc.gpsimd.dma_start(spl[:, GL: 2 * GL], ssp[16:24, :])
                tmp = smallp.tile([H, GL], f32, tag="tmp")
                sl = scoresT[:, GL * g: GL * (g + 1)]
                if masked:
                    nc.vector.tensor_add(tmp[:], ssp[0:H, :], spl[:, 0:GL])
                    tmp2 = smallp.tile([H, GL], f32, tag="tmp2")
                    nc.vector.tensor_add(tmp2[:], tmp[:], spl[:, GL: 2 * GL])
                    in0, in1 = tmp2[:], mb_sb[:, GL * g: GL * (g + 1)]
                else:
                    nc.vector.tensor_add(tmp[:], ssp[0:H, :], spl[:, 0:GL])
                    in0, in1 = tmp[:], spl[:, GL: 2 * GL]
                nc.vector.tensor_add(sl, in0, in1)
                nc.vector.reduce_max(pmax[:, g: g + 1], sl, axis=AX.X)

        def phase_softmax(b):
            scoresT = state[b]["scoresT"]
            pmax = state[b]["pmax"]
            negmax = smallp.tile([H, 1], f32, tag="negmax")
            nc.vector.reduce_max(negmax[:], pmax[:], axis=AX.X, negate=True)
            # top-1 scan on DVE runs in parallel with exp on Act
            r2g = smallp.tile([H, D], f32, tag="r2g")
            mx8 = smallp.tile([H, 8], f32, tag="mx8")
            idx8 = smallp.tile([H, 8], u32, tag="idx8")
            nc.vector.max_with_indices(mx8[:], idx8[:], scoresT[:])
            nc.gpsimd.indirect_dma_start(
                out=r2g[:], out_offset=None,
                in_=r2_d[b],
                in_offset=bass.IndirectOffsetOnAxis(ap=idx8[:, 0:1], axis=0),
            )
            u8 = scp.tile([H, L], fp8, tag="u8")
            NE = 4
            EW = L // NE
            sums = smallp.tile([H, NE], f32, tag="sums")
            for ch in range(NE):
                nc.scalar.activation(
                    u8[:, EW * ch: EW * (ch + 1)],
                    scoresT[:, EW * ch: EW * (ch + 1)],
                    AF.Exp, bias=negmax[:], scale=1.0,
                    accum_out=sums[:, ch: ch + 1],
                )
            stot = smallp.tile([H, 1], f32, tag="stot")
            inv = smallp.tile([H, 1], f32, tag="inv")
            nc.vector.reduce_sum(stot[:], sums[:], axis=AX.X)
            nc.vector.reciprocal(inv[:], stot[:])
            state[b].update(u8=u8, inv=inv, r2g=r2g)

        def phase_pooled(b):
            u8 = state[b]["u8"]
            pd1 = state[b]["pd1"]
            inv = state[b]["inv"]
            r2g = state[b]["r2g"]
            uT = state[b]["uT"]
            uTv = uT[:].rearrange("p (T i m) -> p T i m", T=NT, i=2)
            uv = u8[:].rearrange("h (T l two) -> h T two l", T=NT, two=2)
            for T in range(NT):
                ups = utpp.tile([128, 32], fp8, tag="ups")
                upsv = ups[:].rearrange("p (k two) -> p k two", two=2)
                for rho in range(2):
                    nc.tensor.transpose(
                        upsv[:, 8 * rho: 8 * (rho + 1), 0],
                        uv[:, T, rho],
                        ident8[0:H, 0:H],
                    )
                nc.scalar.copy(
                    uTv[:, T, :, 0:H],
                    upsv[:, 0:16, 0].rearrange("p (i m) -> p i m", i=2),
                )

            # pre-zero + start=False: see scheduling notes on bank zeroing
            pp = ppp.tile([32, D], f32, tag="pp")
            nc.vector.memset(pp[:], 0.0)
            pv = pd1[:].rearrange(
                "p (T cc r q two) -> p T cc r q two", T=NT, cc=NCC, r=2, q=128)
            for T in range(NT):
                for cc in range(NCC):
                    nc.tensor.matmul(
                        pp[:, 256 * cc: 256 * (cc + 1)],
                        uTv[:, T],
                        pv[:, T, cc].rearrange("p r q two -> p two r q"),
                        start=False, stop=(T == NT - 1),
                        perf_mode=DR,
                        skip_group_check=True,
                    )

            pooled = smallp.tile([H, D], f32, tag="pooled")
            nc.vector.tensor_add(pooled[:], pp[0:H, :], r2g[:])
            nc.vector.tensor_scalar_mul(pooled[:], pooled[:], inv[:])
            nc.gpsimd.dma_start(out_d[b], pooled[:])

        compute(0, ETP)
        phase_softmax(0)
        prep(1, 0)
        compute(1, 0)
        phase_pooled(0)
        phase_softmax(1)
        phase_pooled(1)

    nc.compile()
    return nc


def _get_nc(masked: bool):
    if masked not in _CACHE:
        _CACHE[masked] = _build(masked)
    return _CACHE[masked]


def make_in_maps(x, kpm, q, w, masked):
    import ml_dtypes

    fp8np = ml_dtypes.float8_e4m3
    qT = np.ascontiguousarray(np.asarray(q, np.float32).T)
    w = np.ascontiguousarray(np.asarray(w, np.float32))
    x = np.asarray(x, np.float32)

    x1 = x.astype(fp8np)
    r2 = x - x1.astype(np.float32)
    x2s = (64.0 * r2).astype(fp8np)
    r3 = r2 - x2s.astype(np.float32) / 64.0
    x3s = (64.0 * r3).astype(fp8np)

    def sd_pack(xp):
        # (BPC, L, D) fp8 -> (BPC, NG, NCC, 128, 2, 512) bytes
        vv = xp.view(np.uint8).reshape(BPC, NG, GL, NCC, 2, 128)
        return vv.transpose(0, 1, 3, 5, 4, 2)  # b, g, cc, p, r, l

    in_maps = []
    for c in range(NCORES):
        sl = slice(BPC * c, BPC * (c + 1))
        planes = [sd_pack(p[sl]) for p in (x1, x2s, x3s)]
        sd = np.ascontiguousarray(
            np.stack(planes, axis=2)         # b, g, pl, cc, p, r, l
            .transpose(0, 1, 4, 2, 3, 5, 6)  # b, g, p, pl, cc, r, l
        ).reshape(BPC, NG, 128, 3 * NCC * 1024)
        m = {"sd": sd, "qT": qT, "w": w}
        for b in range(BPC):
            m[f"r2_{b}"] = np.ascontiguousarray(r2[BPC * c + b])
        if masked:
            bias = np.where(kpm[sl, None, :], np.float32(-1e30),
                            np.float32(0)).astype(np.float32)
            m["mb"] = np.ascontiguousarray(np.broadcast_to(bias, (BPC, H, L)))
        in_maps.append(m)
    return in_maps


def kernel(**inputs) -> np.ndarray:
    global LAST_RESULTS
    from concourse.bass_utils import run_bass_kernel_spmd

    x = np.asarray(inputs["x"], dtype=np.float32)
    kpm = np.asarray(inputs["kpm"])
    q = np.asarray(inputs["q"], dtype=np.float32)
    w = np.asarray(inputs["w"], dtype=np.float32)

    masked = bool(kpm.any())
    nc = _get_nc(masked)
    in_maps = make_in_maps(x, kpm, q, w, masked)

    trace = bool(os.environ.get("ATTNPOOL_TRACE"))
    res = run_bass_kernel_spmd(nc, in_maps, list(range(NCORES)), trace=trace)
    LAST_RESULTS = res
    out = np.concatenate(
        [r["out"].reshape(BPC, H * D) for r in res.results], axis=0
    )
    return np.ascontiguousarray(out.astype(np.float32))


# revision 40
# speedup vs baseline: 1.1880x; 1.1880x over previous
"""AttnPool Trainium2 kernel (nn_AttnPool_73100343378373).

Math (algebraically identical to the reference):
    scores = (q @ w) @ x.T   per batch  -> (H, L)
    attn   = softmax(scores + mask_bias, axis=L)
    out    = attn @ x  -> (B, H*D)

Distribution: data-parallel over batch, 2 batches per core, q/w replicated.

Precision scheme (fp8 e4m3 on the whole hot path):
  - x is split host-side into three fp8 planes: x ~= x1 + x2s/64 + x3s/64.
  - qw = q@w is computed on device in fp32 (w-stationary matmuls so the
    fp32 4x row cost applies to 8 moving columns only), split into three
    fp8 planes (residual chain) and folded into two 24-wide (padded to
    32) DoubleRow stationaries: S1 = [q1|q2|q3] for the x1 pass, S2 =
    S1/64 for the x2s/x3s passes.  Three DoubleRow score passes per
    group accumulate all 9 cross terms into the same PSUM rows.
  - softmax is near-one-hot (score std ~1200 over L=4096); u = exp fp8.
  - pooled = u @ x1 via DoubleRow matmuls on a pair-layout derived
    on-device from the x1 score plane (bit-exact bf16-bitcast PE
    transposes), plus an exact top-1 correction: the argmax row's
    residual r2 = x - f32(x1) is fetched with an indirect DMA gather
    keyed by on-device max_with_indices, added before normalization.

Scheduling notes (all verified against perfetto traces):
  - ISA: engine (Act/DVE) access patterns must start at 32-aligned
    partitions -> the 3x8-row score PSUM block is realigned via small
    SBUF-to-SBUF DMAs.
  - matmul start=True zeroes the full 2KB PSUM bank for the written
    partitions -> sibling accumulation chains pre-zero with a DVE
    memset and run start=False.
  - SBUF/PSUM pools are created steady-first so the transient stage-0
    pools alias only regions whose first use already depends on
    stage-0 (otherwise the slot-ring DMAs inherit a stage-0 wait).
  - The x1 plane streams in its own slot ring so its pair-transposes
    (which need no stationaries) keep the PE busy during stage 0.
"""

import os
from contextlib import ExitStack

import numpy as np

B, L, D, H = 16, 4096, 1024, 8
NCORES = 8
BPC = B // NCORES  # batches per core
NG = 8             # 512-row L-groups per batch
GL = L // NG       # rows per group = 512
NCC = D // 256     # 256-deep contraction chunks over D = 4
NT = L // 256      # 256-deep contraction chunks over L = 16

VARIANT = {
    "slot_bufs": 4,
    "pd_bufs": 2,
    "tp_bufs": 2,
    "sp_bufs": 3,
    "early_tp": 0,
}

_CACHE: dict = {}
LAST_RESULTS = None  # test harness reads exec_time_ns from here


def _build(masked: bool, variant: dict | None = None):
    import concourse.bass as bass
    import concourse.tile as tile
    from concourse import bacc, mybir
    from concourse.masks import make_identity

    v = dict(VARIANT)
    if variant:
        v.update(variant)
    ETP = v["early_tp"]

    f32 = mybir.dt.float32
    fp8 = mybir.dt.float8e4
    bf16 = mybir.dt.bfloat16
    u32 = mybir.dt.uint32
    AF = mybir.ActivationFunctionType
    AX = mybir.AxisListType
    DR = mybir.MatmulPerfMode.DoubleRow

    nc = bacc.Bacc("TRN2", target_bir_lowering=False, debug=False)

    # sd[b, g, p, (pl, cc, r*512 + l)] = xpl[b, 512 g + l, 256 cc + 128 r + p]
    # per-partition bytes contiguous; plane 0 (bytes 0:4096) is the A-stream.
    sd_d = nc.dram_tensor("sd", (BPC, NG, 128, 3 * NCC * 1024), fp8,
                          kind="ExternalInput").ap()
    qT_d = nc.dram_tensor("qT", (D, H), f32, kind="ExternalInput").ap()
    w_d = nc.dram_tensor("w", (D, D), f32, kind="ExternalInput").ap()
    r2_d = [nc.dram_tensor(f"r2_{b}", (L, D), f32, kind="ExternalInput").ap()
            for b in range(BPC)]
    if masked:
        mb_d = nc.dram_tensor("mb", (BPC, H, L), f32, kind="ExternalInput").ap()
    out_d = nc.dram_tensor("out", (BPC, H, D), f32, kind="ExternalOutput").ap()

    with tile.TileContext(nc) as tc, ExitStack() as ctx:
        const = ctx.enter_context(tc.tile_pool(name="const", bufs=1))

        ident = const.tile([128, 128], bf16, tag="ident")
        make_identity(nc, ident[:])
        ident8 = const.tile([128, 128], fp8, tag="ident8")
        nc.vector.tensor_copy(ident8[:], ident[:])
        identf = const.tile([128, 128], f32, tag="identf")
        nc.vector.tensor_copy(identf[:], ident[:])

        # steady pools first (see scheduling notes)
        slotp = ctx.enter_context(tc.tile_pool(name="slot", bufs=v["slot_bufs"]))
        pdp = ctx.enter_context(tc.tile_pool(name="pd", bufs=v["pd_bufs"]))
        scp = ctx.enter_context(tc.tile_pool(name="sc", bufs=2))
        smallp = ctx.enter_context(tc.tile_pool(name="small", bufs=2))
        tpp = ctx.enter_context(tc.tile_pool(name="tp", bufs=v["tp_bufs"], space="PSUM"))
        utpp = ctx.enter_context(tc.tile_pool(name="utps", bufs=1, space="PSUM"))

        st_tiles = [[None] * NCC for _ in range(2)]
        state = {}

        def prep(b):
            """Allocate the batch's pd1/uT tiles and zero the uT pads."""
            pd1 = pdp.tile([128, NT * 2048], fp8, tag="pd1")
            uT = smallp.tile([128, NT * 64], fp8, tag="uT")
            nc.gpsimd.memset(uT[:], 0.0)
            state[b] = dict(pd1=pd1, uT=uT, slots=[])

        def emit_tp(b, g):
            """Pair-transpose the x1 plane of group g into the pooled DR
            layout (bf16 bitcast, bit-exact)."""
            pd1 = state[b]["pd1"]
            a = state[b]["slots"][g]
            tv = a[:].rearrange("p (pl c r t y) -> p pl c r t y",
                                pl=3, c=NCC, r=2, t=2)[:, 0]
            for t in range(2):
                tp = tpp.tile([128, 1024], bf16, tag="tp")
                for cc in range(NCC):
                    for r in range(2):
                        nc.tensor.transpose(
                            tp[:, 128 * (2 * cc + r): 128 * (2 * cc + r + 1)],
                            tv[:, cc, r, t].bitcast(bf16),
                            ident[:],
                        )
                dst = pd1[:].bitcast(bf16).rearrange(
                    "p (T y) -> p T y", T=NT)[:, 2 * g + t]
                if (g + t) % 2 == 0:
                    nc.scalar.copy(dst, tp[:])
                else:
                    nc.vector.tensor_copy(dst, tp[:])

        # ---- stage 0: qw = q @ w, plane split, DR stationaries ----------
        with tc.tile_pool(name="s0", bufs=1) as s0p, \
             tc.tile_pool(name="s0w", bufs=2) as s0wp, \
             tc.tile_pool(name="s0ps", bufs=1, space="PSUM") as s0ps:
            qT_sb = const.tile([128, 64], f32, tag="qT")
            nc.gpsimd.dma_start(
                qT_sb[:].rearrange("p (c h) -> p c h", c=8),
                qT_d.rearrange("(c p) h -> p c h", p=128),
            )
            # qw^T via w-stationary matmuls: moving operand is qT (8 cols),
            # so the fp32 4x row cost applies to only 8 columns per matmul.
            qwT_ps = s0ps.tile([128, 64], f32, tag="qwtps")
            nc.vector.memset(qwT_ps[:], 0.0)
            for dc in range(8):
                for wh in range(2):
                    w_t = s0wp.tile([128, 512], f32, tag="w")
                    nc.sync.dma_start(
                        w_t[:], w_d[128 * dc: 128 * (dc + 1),
                                    512 * wh: 512 * (wh + 1)])
                    for fbh in range(4):
                        fb = 4 * wh + fbh
                        nc.tensor.matmul(
                            qwT_ps[:, 8 * fb: 8 * (fb + 1)],
                            w_t[:, 128 * fbh: 128 * (fbh + 1)],
                            qT_sb[:, 8 * dc: 8 * (dc + 1)],
                            start=False, stop=(dc == 7),
                            skip_group_check=True,
                        )

            # batch 0 DMAs + early transposes keep DMA and PE busy while
            # the stationary chain below resolves.
            prep(0)

            qwT_sb = s0p.tile([128, 64], f32, tag="qwT")
            nc.scalar.copy(qwT_sb[:], qwT_ps[:])
            qw_sb = s0p.tile([8, D], f32, tag="qw")
            for fb in range(8):
                qw2b = s0ps.tile([8, 512], f32, tag="qw2b", bufs=2)
                nc.tensor.transpose(
                    qw2b[:, 0:128],
                    qwT_sb[:, 8 * fb: 8 * (fb + 1)],
                    identf[:],
                )
                nc.scalar.copy(qw_sb[:, 128 * fb: 128 * (fb + 1)],
                               qw2b[:, 0:128])

            # residual-chain plane split, in place: qw_sb becomes the
            # running residual; p8[(0, r)] are the canonical fp8 planes.
            p8 = {}
            for r in range(3):
                for si in range(2):
                    p8[(si, r)] = s0p.tile([8, D], fp8, tag=f"p8_{si}_{r}",
                                           name=f"p8_{si}_{r}")
            q1f = s0p.tile([8, D], f32, tag="q1f")
            nc.scalar.copy(p8[(0, 0)][:], qw_sb[:])
            nc.scalar.copy(q1f[:], p8[(0, 0)][:])
            nc.vector.tensor_sub(qw_sb[:], qw_sb[:], q1f[:])
            nc.scalar.copy(p8[(0, 1)][:], qw_sb[:])
            nc.scalar.copy(q1f[:], p8[(0, 1)][:])
            nc.vector.tensor_sub(qw_sb[:], qw_sb[:], q1f[:])
            nc.scalar.copy(p8[(0, 2)][:], qw_sb[:])
            for r in range(3):
                nc.scalar.activation(p8[(1, r)][:], p8[(0, r)][:],
                                     AF.Copy, scale=1.0 / 64.0)
            for si in range(2):
                for cc in range(NCC):
                    ps = s0ps.tile([128, 2048], fp8, tag="stps", bufs=1)
                    psv = ps[:].rearrange("p (k two) -> p k two", two=2)
                    for i in range(2):
                        for r in range(3):
                            nc.tensor.transpose(
                                psv[:, 32 * i + 8 * r: 32 * i + 8 * r + 8, 0],
                                p8[(si, r)][:, 256 * cc + 128 * i:
                                             256 * cc + 128 * (i + 1)],
                                ident8[0:H, 0:H],
                            )
                    st = const.tile([128, 64], fp8, tag=f"st{si}_{cc}")
                    nc.vector.tensor_copy(st[:], psv[:, 0:64, 0])
                    for i in range(2):
                        nc.gpsimd.memset(st[:, 32 * i + 24: 32 * (i + 1)], 0.0)
                    st_tiles[si][cc] = st

        # PSUM pools that alias the (dead) stage-0 psum: their first use
        # already depends on stage-0 output.
        spp = ctx.enter_context(tc.tile_pool(name="sp", bufs=v["sp_bufs"], space="PSUM"))
        ppp = ctx.enter_context(tc.tile_pool(name="pp", bufs=1, space="PSUM"))

        def compute(b):
            if masked:
                mb_sb = scp.tile([H, L], f32, tag="mb", bufs=1)
                nc.gpsimd.dma_start(mb_sb[:], mb_d[b])
            scoresT = scp.tile([H, L], f32, tag="scoresT")
            pmax = smallp.tile([H, NG], f32, tag="pmax")
            state[b].update(scoresT=scoresT, pmax=pmax)
            for g in range(NG):
                slot = slotp.tile([128, 3 * NCC * 1024], fp8, tag="slot")
                sdv = sd_d[b, g].rearrange("p (pl y) -> p pl y", pl=3)
                slv = slot[:].rearrange("p (pl y) -> p pl y", pl=3)
                for pl in range(3):
                    nc.sync.dma_start(slv[:, pl], sdv[:, pl])
                state[b]["slots"].append(slot)
                sv = slot[:].rearrange(
                    "p (pl c r l) -> p pl c r l", pl=3, c=NCC, r=2)
                sp = spp.tile([32, GL], f32, tag="sp")
                for pl in range(3):
                    for cc in range(NCC):
                        nc.tensor.matmul(
                            sp[:, :],
                            st_tiles[0 if pl == 0 else 1][cc][:].rearrange(
                                "p (i m) -> p i m", i=2),
                            sv[:, pl, cc],
                            start=(pl == 0 and cc == 0),
                            stop=(pl == 2 and cc == NCC - 1),
                            perf_mode=DR,
                            skip_group_check=True,
                        )
                emit_tp(b, g)

                # engine APs must start at 32-aligned partitions: copy the
                # 32-row psum block out, then DMA-realign rows 8:24.
                ssp = smallp.tile([32, GL], f32, tag="ssp")
                nc.scalar.copy(ssp[:], sp[:, :])
                spl = smallp.tile([H, 2 * GL], f32, tag="spl")
                nc.gpsimd.dma_start(spl[:, 0:GL], ssp[8:16, :])
                nc.gpsimd.dma_start(spl[:, GL: 2 * GL], ssp[16:24, :])
                tmp = smallp.tile([H, GL], f32, tag="tmp")
                sl = scoresT[:, GL * g: GL * (g + 1)]
                if masked:
                    nc.vector.tensor_add(tmp[:], ssp[0:H, :], spl[:, 0:GL])
                    tmp2 = smallp.tile([H, GL], f32, tag="tmp2")
                    nc.vector.tensor_add(tmp2[:], tmp[:], spl[:, GL: 2 * GL])
                    in0, in1 = tmp2[:], mb_sb[:, GL * g: GL * (g + 1)]
                else:
                    nc.vector.tensor_add(tmp[:], ssp[0:H, :], spl[:, 0:GL])
                    in0, in1 = tmp[:], spl[:, GL: 2 * GL]
                nc.vector.tensor_add(sl, in0, in1)
                nc.vector.reduce_max(pmax[:, g: g + 1], sl, axis=AX.X)

        def phase_softmax(b):
            scoresT = state[b]["scoresT"]
            pmax = state[b]["pmax"]
            negmax = smallp.tile([H, 1], f32, tag="negmax")
            nc.vector.reduce_max(negmax[:], pmax[:], axis=AX.X, negate=True)
            # top-1 scan on DVE runs in parallel with exp on Act
            r2g = smallp.tile([H, D], f32, tag="r2g")
            mx8 = smallp.tile([H, 8], f32, tag="mx8")
            idx8 = smallp.tile([H, 8], u32, tag="idx8")
            nc.vector.max_with_indices(mx8[:], idx8[:], scoresT[:])
            nc.gpsimd.indirect_dma_start(
                out=r2g[:], out_offset=None,
                in_=r2_d[b],
                in_offset=bass.IndirectOffsetOnAxis(ap=idx8[:, 0:1], axis=0),
            )
            u8 = scp.tile([H, L], fp8, tag="u8")
            NE = 4
            EW = L // NE
            sums = smallp.tile([H, NE], f32, tag="sums")
            for ch in range(NE):
                nc.scalar.activation(
                    u8[:, EW * ch: EW * (ch + 1)],
                    scoresT[:, EW * ch: EW * (ch + 1)],
                    AF.Exp, bias=negmax[:], scale=1.0,
                    accum_out=sums[:, ch: ch + 1],
                )
            stot = smallp.tile([H, 1], f32, tag="stot")
            inv = smallp.tile([H, 1], f32, tag="inv")
            nc.vector.reduce_sum(stot[:], sums[:], axis=AX.X)
            nc.vector.reciprocal(inv[:], stot[:])
            state[b].update(u8=u8, inv=inv, r2g=r2g)

        def phase_pooled(b):
            u8 = state[b]["u8"]
            pd1 = state[b]["pd1"]
            inv = state[b]["inv"]
            r2g = state[b]["r2g"]
            uT = state[b]["uT"]
            uTv = uT[:].rearrange("p (T i m) -> p T i m", T=NT, i=2)
            uv = u8[:].rearrange("h (T l two) -> h T two l", T=NT, two=2)
            for T in range(NT):
                ups = utpp.tile([128, 32], fp8, tag="ups")
                upsv = ups[:].rearrange("p (k two) -> p k two", two=2)
                for rho in range(2):
                    nc.tensor.transpose(
                        upsv[:, 8 * rho: 8 * (rho + 1), 0],
                        uv[:, T, rho],
                        ident8[0:H, 0:H],
                    )
                nc.scalar.copy(
                    uTv[:, T, :, 0:H],
                    upsv[:, 0:16, 0].rearrange("p (i m) -> p i m", i=2),
                )

            # pre-zero + start=False: see scheduling notes on bank zeroing
            pp = ppp.tile([32, D], f32, tag="pp")
            nc.vector.memset(pp[:], 0.0)
            pv = pd1[:].rearrange(
                "p (T cc r q two) -> p T cc r q two", T=NT, cc=NCC, r=2, q=128)
            for T in range(NT):
                for cc in range(NCC):
                    nc.tensor.matmul(
                        pp[:, 256 * cc: 256 * (cc + 1)],
                        uTv[:, T],
                        pv[:, T, cc].rearrange("p r q two -> p two r q"),
                        start=False, stop=(T == NT - 1),
                        perf_mode=DR,
                        skip_group_check=True,
                    )

            pooled = smallp.tile([H, D], f32, tag="pooled")
            nc.vector.tensor_add(pooled[:], pp[0:H, :], r2g[:])
            nc.vector.tensor_scalar_mul(pooled[:], pooled[:], inv[:])
            nc.gpsimd.dma_start(out_d[b], pooled[:])

        compute(0)
        phase_softmax(0)
        prep(1)
        compute(1)
        phase_pooled(0)
        phase_softmax(1)
        phase_pooled(1)

    nc.compile()
    return nc


def _get_nc(masked: bool):
    if masked not in _CACHE:
        _CACHE[masked] = _build(masked)
    return _CACHE[masked]


def make_in_maps(x, kpm, q, w, masked):
    import ml_dtypes

    fp8np = ml_dtypes.float8_e4m3
    qT = np.ascontiguousarray(np.asarray(q, np.float32).T)
    w = np.ascontiguousarray(np.asarray(w, np.float32))
    x = np.asarray(x, np.float32)

    x1 = x.astype(fp8np)
    r2 = x - x1.astype(np.float32)
    x2s = (64.0 * r2).astype(fp8np)
    r3 = r2 - x2s.astype(np.float32) / 64.0
    x3s = (64.0 * r3).astype(fp8np)

    def sd_pack(xp):
        # (BPC, L, D) fp8 -> (BPC, NG, NCC, 128, 2, 512) bytes
        vv = xp.view(np.uint8).reshape(BPC, NG, GL, NCC, 2, 128)
        return vv.transpose(0, 1, 3, 5, 4, 2)  # b, g, cc, p, r, l

    in_maps = []
    for c in range(NCORES):
        sl = slice(BPC * c, BPC * (c + 1))
        planes = [sd_pack(p[sl]) for p in (x1, x2s, x3s)]
        sd = np.ascontiguousarray(
            np.stack(planes, axis=2)         # b, g, pl, cc, p, r, l
            .transpose(0, 1, 4, 2, 3, 5, 6)  # b, g, p, pl, cc, r, l
        ).reshape(BPC, NG, 128, 3 * NCC * 1024)
        m = {"sd": sd, "qT": qT, "w": w}
        for b in range(BPC):
            m[f"r2_{b}"] = np.ascontiguousarray(r2[BPC * c + b])
        if masked:
            bias = np.where(kpm[sl, None, :], np.float32(-1e30),
                            np.float32(0)).astype(np.float32)
            m["mb"] = np.ascontiguousarray(np.broadcast_to(bias, (BPC, H, L)))
        in_maps.append(m)
    return in_maps


def kernel(**inputs) -> np.ndarray:
    global LAST_RESULTS
    from concourse.bass_utils import run_bass_kernel_spmd

    x = np.asarray(inputs["x"], dtype=np.float32)
    kpm = np.asarray(inputs["kpm"])
    q = np.asarray(inputs["q"], dtype=np.float32)
    w = np.asarray(inputs["w"], dtype=np.float32)

    masked = bool(kpm.any())
    nc = _get_nc(masked)
    in_maps = make_in_maps(x, kpm, q, w, masked)

    trace = bool(os.environ.get("ATTNPOOL_TRACE"))
    res = run_bass_kernel_spmd(nc, in_maps, list(range(NCORES)), trace=trace)
    LAST_RESULTS = res
    out = np.concatenate(
        [r["out"].reshape(BPC, H * D) for r in res.results], axis=0
    )
    return np.ascontiguousarray(out.astype(np.float32))
